# revision 5
# baseline (speedup 1.0000x reference)
"""Multi-head self-attention on 8 Trainium2 NeuronCores — v2.

Problem: B=4, S=2048, D=1024, H=16 heads (dk=64), torch-Linear style
projections (y = x @ W.T + b), softmax attention, output projection.

Sharding: 8 cores = 4 batches x 2 head-groups (8 heads each).  Per core:
    QT = (Wq_g/sqrt(dk)) @ x_b.T (+bq via K=1 ones-matmul)  [512, S]
    KT = Wk_g @ x_b.T            [512, S]  (bk dropped: cancels in softmax)
    V  = x_b @ Wv_g.T + bv       [S, 512]
    per head-pair pr: scoresT = K_h @ Q_h.T  as TWO concurrent 64-row
    PE tiles (row tiling: head even on partitions 0:64 / tile (0,0),
    head odd on 64:128 / tile (64,0)) writing separate PSUM banks.
    exp: head-even tiles on the Scalar engine (exact exp), a tunable
    subset of head-odd tiles on the Vector engine via a one-instruction
    Schraudolph fast-exp (bits = int16(s*184.66496 + 16249.5) viewed as
    bf16; ~1.8% rms per affected tile, diluted by the subset fraction).
    PV via the ones-columns trick ([V_h|ones] / [ones|V_h] stationaries)
    so softmax denominators fall out of the same matmuls.
    partialT = Wo_g @ onorm      [1024, S]
Host sums the two group partials per batch, transposes, adds bo.

Perf notes vs v1: query stripes of 1024 (PSUM: s0,s1,pv0,pv1 = 2 banks
each), scores matmuls emitted pairwise so LDWEIGHTS for one 64-row tile
pulls ahead under the other tile's in-flight matmul, exp offloaded
partially to DVE, PSUM evictions split ACT/DVE, and a post-schedule
pass that drops InstLdweights reloading the stationary operand already
resident in the same PE array tile.
"""

import math

import numpy as np
import ml_dtypes

import concourse.bass as bass
import concourse.bacc as bacc_mod
import concourse.mybir as mybir
import concourse.tile as tile
from concourse.bass_utils import run_bass_kernel_spmd

BF16 = mybir.dt.bfloat16
F32 = mybir.dt.float32
I16 = mybir.dt.int16
AF = mybir.ActivationFunctionType
ALU = mybir.AluOpType

B, S, D, H = 4, 2048, 1024, 16
DK = D // H  # 64
NCORES = 8
GROUPS = 2
DG = D // GROUPS  # 512
P = 128
FT = DG // P  # 4 head pairs per group
KB = S // P  # 16 key blocks
DKB = D // P  # 8 contraction blocks
QH = 1024  # query stripe
NQH = S // QH  # 2
QC = 512  # matmul free-dim chunk
NQC_H = QH // QC  # 2
NQC = S // QC  # 4

# Schraudolph bf16 fast-exp constants (validated on HW: rms 1.8%/tile)
FE_SCALE = 184.66496  # 2^7 * log2(e)
FE_BIAS = 16249.5  # 127*2^7 - 6.5 (error centering)
# head-odd kb tiles handled by DVE fast-exp (rest on ACT exact exp)
DVE_KBS = frozenset({1, 3, 5, 7, 9, 12, 14})

DEDUP_LDW = True


class _Bacc(bacc_mod.Bacc):
    """Keep semaphore waits on matmuls (split into event-semaphores
    later) instead of migrating them onto ldweights — required for the
    ldweights dedup pass to stay hazard-free."""

    def move_matmul_waits_to_ldweights(self):
        pass


def _row_range(inst):
    tp = inst.tile_position
    ts = inst.tile_size
    r0 = tp[0] if tp is not None else 0
    rn = ts[0] if ts is not None else 128
    return (r0, r0 + rn)


def dedup_ldweights(nc) -> int:
    """Remove InstLdweights whose weights are already resident in the
    targeted PE-array tile (same AP, same tile position/size, no
    row-overlapping load in between).  Only wait-free loads are dropped,
    so synchronization is untouched.  The simulator does not model PE
    weight state; hardware rel-err validates this pass."""
    removed = 0
    for f in nc.m.functions:
        for blk in f.blocks:
            insts = list(blk.instructions)
            keep = []
            changed = False
            loaded: dict = {}  # key -> (sig, row_range)
            for inst in insts:
                if type(inst).__name__ == "InstLdweights":
                    key = (str(inst.tile_position), str(inst.tile_size))
                    sig = str(inst.ins[0])
                    has_wait = bool(inst.sync_info and inst.sync_info.on_wait)
                    prev = loaded.get(key)
                    if not has_wait and prev is not None and prev[0] == sig:
                        removed += 1
                        changed = True
                        continue
                    rr = _row_range(inst)
                    for k in list(loaded):
                        if k != key and _ranges_overlap(loaded[k][1], rr):
                            del loaded[k]
                    loaded[key] = (sig, rr)
                keep.append(inst)
            if changed:
                blk.instructions = keep
    return removed


def _ranges_overlap(a, b):
    return a[0] < b[1] and b[0] < a[1]


def build_attention_nc(seq: int = S) -> bass.Bass:
    kb_n = seq // P
    nqh = seq // QH

    nc = _Bacc("TRN2", num_devices=NCORES)
    xt_d = nc.declare_dram_parameter("xt", [D, seq], BF16, isOutput=False)
    wqt_d = nc.declare_dram_parameter("wqt", [D, DG], BF16, isOutput=False)
    wkt_d = nc.declare_dram_parameter("wkt", [D, DG], BF16, isOutput=False)
    wvt_d = nc.declare_dram_parameter("wvt", [D, DG], BF16, isOutput=False)
    wot_d = nc.declare_dram_parameter("wot", [DG, D], BF16, isOutput=False)
    bq_d = nc.declare_dram_parameter("bqr", [1, DG], BF16, isOutput=False)
    bv_d = nc.declare_dram_parameter("bvr", [1, DG], BF16, isOutput=False)
    out_d = nc.declare_dram_parameter("out", [D, seq], F32, isOutput=True)

    with tile.TileContext(nc) as tc:
        with tc.tile_pool(name="persist", bufs=1) as persist:
            ones_bf = persist.tile([1, QC], BF16, name="ones_bf")
            nc.vector.memset(ones_bf, 1.0)
            bq_sb = persist.tile([1, DG], BF16, name="bq_sb")
            nc.sync.dma_start(bq_sb, bq_d[:, :])
            bv_sb = persist.tile([1, DG], BF16, name="bv_sb")
            nc.sync.dma_start(bv_sb, bv_d[:, :])

            qt_sb = [persist.tile([P, seq], BF16, name=f"qt{i}") for i in range(FT)]
            kt_sb = [persist.tile([P, seq], BF16, name=f"kt{i}") for i in range(FT)]
            v2_sb = [persist.tile([P, 2 * DG], BF16, name=f"v{i}") for i in range(kb_n)]
            wot_sb = [persist.tile([P, D], BF16, name=f"wot{i}") for i in range(FT)]
            onorm = [persist.tile([P, seq], BF16, name=f"onorm{i}") for i in range(FT)]
            for ft in range(FT):
                nc.sync.dma_start(wot_sb[ft], wot_d[ft * P : (ft + 1) * P, :])

            # ---------------- phase 1: projections ----------------
            with tc.tile_pool(name="xw", bufs=1) as xw_pool:
                xt_sb = []
                wq_sb = []
                wk_sb = []
                wv_sb = []
                for i in range(DKB):
                    wq_i = xw_pool.tile([P, DG], BF16, name=f"wqts{i}")
                    nc.sync.dma_start(wq_i, wqt_d[i * P : (i + 1) * P, :])
                    wq_sb.append(wq_i)
                    wk_i = xw_pool.tile([P, DG], BF16, name=f"wkts{i}")
                    nc.sync.dma_start(wk_i, wkt_d[i * P : (i + 1) * P, :])
                    wk_sb.append(wk_i)
                    wv_i = xw_pool.tile([P, DG], BF16, name=f"wvts{i}")
                    nc.sync.dma_start(wv_i, wvt_d[i * P : (i + 1) * P, :])
                    wv_sb.append(wv_i)
                    xti = xw_pool.tile([P, seq], BF16, name=f"xts{i}")
                    nc.sync.dma_start(xti, xt_d[i * P : (i + 1) * P, :])
                    xt_sb.append(xti)

                # Q/K: features on partitions; lhsT reused across chunks
                with tc.tile_pool(name="qkps", bufs=1, space="PSUM") as qk_ps:
                  for pr in range(FT):
                    fsl = slice(pr * P, (pr + 1) * P)
                    psq = [
                        qk_ps.tile([P, QC], F32, name=f"psq{c}", tag=f"psq{c}")
                        for c in range(NQC)
                    ]
                    psk = [
                        qk_ps.tile([P, QC], F32, name=f"psk{c}", tag=f"psk{c}")
                        for c in range(NQC)
                    ]
                    # bias seeds the Q accumulation (per-feature bias via
                    # K=1 matmul: out[f,q] += bq[f] * 1)
                    for c in range(NQC):
                        nc.tensor.matmul(
                            psq[c], lhsT=bq_sb[0:1, fsl], rhs=ones_bf,
                            start=True, stop=False,
                        )
                    for k in range(DKB):
                        for c in range(NQC):
                            csl = slice(c * QC, (c + 1) * QC)
                            nc.tensor.matmul(
                                psq[c], lhsT=wq_sb[k][:, fsl],
                                rhs=xt_sb[k][:, csl],
                                start=False, stop=k == DKB - 1,
                            )
                        for c in range(NQC):
                            csl = slice(c * QC, (c + 1) * QC)
                            nc.tensor.matmul(
                                psk[c], lhsT=wk_sb[k][:, fsl],
                                rhs=xt_sb[k][:, csl],
                                start=k == 0, stop=k == DKB - 1,
                            )
                    for c in range(NQC):
                        csl = slice(c * QC, (c + 1) * QC)
                        nc.scalar.activation(qt_sb[pr][:, csl], psq[c], AF.Identity)
                        nc.scalar.activation(kt_sb[pr][:, csl], psk[c], AF.Identity)

                # V: keys on partitions, features free; bias via K=1 matmul
                with tc.tile_pool(name="vps", bufs=4, space="PSUM") as v_ps:
                    for kb in range(kb_n):
                        ksl = slice(kb * P, (kb + 1) * P)
                        psv = v_ps.tile([P, DG], F32, name="psv", tag="psv")
                        nc.tensor.matmul(
                            psv, lhsT=ones_bf[0:1, 0:P], rhs=bv_sb,
                            start=True, stop=False,
                        )
                        for k in range(DKB):
                            nc.tensor.matmul(
                                psv, lhsT=xt_sb[k][:, ksl], rhs=wv_sb[k],
                                start=False, stop=k == DKB - 1,
                            )
                        nc.vector.memset(v2_sb[kb], 1.0)
                        # even heads -> cols [256q, 0:64); odd -> [256q+192, +256)
                        nc.vector.tensor_copy(
                            v2_sb[kb].rearrange("p (q c) -> p q c", c=256)[:, :, 0:64],
                            psv.rearrange("p (q c) -> p q c", c=128)[:, :, 0:64],
                        )
                        nc.vector.tensor_copy(
                            v2_sb[kb].rearrange("p (q c) -> p q c", c=256)[:, :, 192:256],
                            psv.rearrange("p (q c) -> p q c", c=128)[:, :, 64:128],
                        )

            # ---------------- phase 2: attention ----------------
            with (
                tc.tile_pool(name="sps", bufs=1, space="PSUM") as s_ps,
                tc.tile_pool(name="pvps", bufs=1, space="PSUM") as pv_ps,
                tc.tile_pool(name="epool", bufs=3) as e_pool,
                tc.tile_pool(name="mpool", bufs=2) as m_pool,
            ):
                for pr in range(FT):
                    h0c = slice((2 * pr) * P, (2 * pr + 1) * P)
                    h1c = slice((2 * pr + 1) * P, (2 * pr + 2) * P)
                    for qh in range(nqh):
                        qbase = qh * QH
                        pv0 = pv_ps.tile([P, QH], F32, name="pv0", tag="pv0")
                        pv1 = pv_ps.tile([P, QH], F32, name="pv1", tag="pv1")
                        for kb in range(kb_n):
                            ksl = slice(kb * P, (kb + 1) * P)
                            s0 = s_ps.tile([P, QH], F32, name="s0", tag="s0")
                            s1 = s_ps.tile([P, QH], F32, name="s1", tag="s1")
                            # paired emission: LDW of one 64-row tile pulls
                            # ahead under the other tile's matmul
                            for c in range(NQC_H):
                                qsl = slice(qbase + c * QC, qbase + (c + 1) * QC)
                                osl = slice(c * QC, (c + 1) * QC)
                                nc.tensor.matmul(
                                    s0[:, osl], lhsT=kt_sb[pr][0:64, ksl],
                                    rhs=qt_sb[pr][0:64, qsl],
                                    start=True, stop=True,
                                )
                                nc.tensor.matmul(
                                    s1[:, osl], lhsT=kt_sb[pr][64:128, ksl],
                                    rhs=qt_sb[pr][64:128, qsl],
                                    start=True, stop=True,
                                )
                            e0 = e_pool.tile([P, QH], BF16, name="e0", tag="e0")
                            e1 = e_pool.tile([P, QH], BF16, name="e1", tag="e1")
                            nc.scalar.activation(e0, s0, AF.Exp)
                            if kb in DVE_KBS:
                                nc.vector.tensor_scalar(
                                    e1.bitcast(I16), s1, FE_SCALE, FE_BIAS,
                                    ALU.mult, ALU.add,
                                )
                            else:
                                nc.scalar.activation(e1, s1, AF.Exp)
                            first, last = kb == 0, kb == kb_n - 1
                            for c in range(NQC_H):
                                osl = slice(c * QC, (c + 1) * QC)
                                nc.tensor.matmul(
                                    pv0[:, osl], lhsT=v2_sb[kb][:, h0c],
                                    rhs=e0[:, osl], start=first, stop=last,
                                )
                            for c in range(NQC_H):
                                osl = slice(c * QC, (c + 1) * QC)
                                nc.tensor.matmul(
                                    pv1[:, osl], lhsT=v2_sb[kb][:, h1c],
                                    rhs=e1[:, osl], start=first, stop=last,
                                )
                        # epilogue: evacuate pv (frees banks), then swap
                        # denominator halves across partitions via DMA,
                        # reciprocal, normalize.
                        # pv0 rows 0:64 outT_h0 / 64:128 den_h0 (ones cols)
                        # pv1 rows 0:64 den_h1  / 64:128 outT_h1
                        pvs0 = m_pool.tile([P, QH], F32, name="pvs0", tag="pvs0")
                        pvs1 = m_pool.tile([P, QH], F32, name="pvs1", tag="pvs1")
                        nc.scalar.activation(pvs0, pv0, AF.Identity)
                        nc.scalar.activation(pvs1, pv1, AF.Identity)
                        dsw = m_pool.tile([P, QH], F32, name="dsw", tag="dsw")
                        nc.sync.dma_start(dsw[0:64, :], pvs0[64:128, :])
                        nc.sync.dma_start(dsw[64:128, :], pvs1[0:64, :])
                        rec = m_pool.tile([P, QH], F32, name="rec", tag="rec")
                        nc.vector.reciprocal_approx_fast(rec, dsw)
                        qful = slice(qbase, qbase + QH)
                        nc.vector.tensor_tensor(
                            onorm[pr][0:64, qful], pvs0[0:64, :], rec[0:64, :],
                            ALU.mult,
                        )
                        nc.vector.tensor_tensor(
                            onorm[pr][64:128, qful], pvs1[64:128, :], rec[64:128, :],
                            ALU.mult,
                        )

            # ---------------- phase 3: output projection ----------------
            with (
                tc.tile_pool(name="ops", bufs=2, space="PSUM") as o_ps,
                tc.tile_pool(name="osb", bufs=8) as o_sb_pool,
            ):
                for dt in range(DKB):
                    dsl = slice(dt * P, (dt + 1) * P)
                    pso = [
                        o_ps.tile([P, QC], F32, name=f"pso{c}", tag=f"pso{c}")
                        for c in range(NQC)
                    ]
                    for ft in range(FT):
                        for c in range(NQC):
                            csl = slice(c * QC, (c + 1) * QC)
                            nc.tensor.matmul(
                                pso[c], lhsT=wot_sb[ft][:, dsl],
                                rhs=onorm[ft][:, csl],
                                start=ft == 0, stop=ft == FT - 1,
                            )
                    for c in range(NQC):
                        csl = slice(c * QC, (c + 1) * QC)
                        o_sb = o_sb_pool.tile([P, QC], F32, name="o_sb", tag="osb")
                        if c % 2 == 0:
                            nc.scalar.activation(o_sb, pso[c], AF.Identity)
                        else:
                            nc.vector.tensor_copy(o_sb, pso[c])
                        nc.sync.dma_start(out_d[dsl, csl], o_sb)

    return nc


_CACHE: dict = {}


def _get_nc(seq: int = S) -> bass.Bass:
    key = f"nc{seq}"
    if key not in _CACHE:
        nc = build_attention_nc(seq)
        if DEDUP_LDW:
            dedup_ldweights(nc)
        nc.finalize()
        _CACHE[key] = nc
    return _CACHE[key]


def make_in_maps(x, Wq, bq, Wk, Wv, bv, Wo, seq: int = S):
    bf = ml_dtypes.bfloat16
    scale = 1.0 / math.sqrt(DK)
    x = np.asarray(x, np.float32)
    Wq = np.asarray(Wq, np.float32)
    bq = np.asarray(bq, np.float32)
    Wk = np.asarray(Wk, np.float32)
    Wv = np.asarray(Wv, np.float32)
    bv = np.asarray(bv, np.float32)
    Wo = np.asarray(Wo, np.float32)
    in_maps = []
    for core in range(NCORES):
        b, g = divmod(core, GROUPS)
        gsl = slice(g * DG, (g + 1) * DG)
        in_maps.append(
            {
                "xt": np.ascontiguousarray(x[b, :seq, :].T).astype(bf),
                "wqt": np.ascontiguousarray((Wq[gsl, :] * scale).T).astype(bf),
                "wkt": np.ascontiguousarray(Wk[gsl, :].T).astype(bf),
                "wvt": np.ascontiguousarray(Wv[gsl, :].T).astype(bf),
                "wot": np.ascontiguousarray(Wo[:, gsl].T).astype(bf),
                "bqr": (bq[gsl] * scale).astype(bf).reshape(1, DG),
                "bvr": bv[gsl].astype(bf).reshape(1, DG),
            }
        )
    return in_maps


def run_device(in_maps, seq: int = S, trace: bool = False):
    nc = _get_nc(seq)
    return run_bass_kernel_spmd(nc, in_maps, list(range(NCORES)), trace=trace)


def kernel(x, Wq, bq, Wk, bk, Wv, bv, Wo, bo):
    in_maps = make_in_maps(x, Wq, bq, Wk, Wv, bv, Wo)
    res = run_device(in_maps).results
    bo = np.asarray(bo, np.float32)
    out = np.empty((B, S, D), np.float32)
    for b in range(B):
        acc = res[2 * b]["out"].astype(np.float32) + res[2 * b + 1]["out"].astype(
            np.float32
        )
        out[b] = acc.T + bo[None, :]
    return out


# revision 7
# speedup vs baseline: 1.2847x; 1.2847x over previous
"""Multi-head self-attention on 8 Trainium2 NeuronCores — v2.

Problem: B=4, S=2048, D=1024, H=16 heads (dk=64), torch-Linear style
projections (y = x @ W.T + b), softmax attention, output projection.

Sharding: 8 cores = 4 batches x 2 head-groups (8 heads each).  Per core:
    QT = (Wq_g/sqrt(dk)) @ x_b.T (+bq via K=1 ones-matmul)  [512, S]
    KT = Wk_g @ x_b.T            [512, S]  (bk dropped: cancels in softmax)
    V  = x_b @ Wv_g.T + bv       [S, 512]
    per head-pair pr: scoresT = K_h @ Q_h.T  as TWO concurrent 64-row
    PE tiles (row tiling: head even on partitions 0:64 / tile (0,0),
    head odd on 64:128 / tile (64,0)) writing separate PSUM banks.
    exp: head-even tiles on the Scalar engine (exact exp), a tunable
    subset of head-odd tiles on the Vector engine via a one-instruction
    Schraudolph fast-exp (bits = int16(s*184.66496 + 16249.5) viewed as
    bf16; ~1.8% rms per affected tile, diluted by the subset fraction).
    PV via the ones-columns trick ([V_h|ones] / [ones|V_h] stationaries)
    so softmax denominators fall out of the same matmuls.
    partialT = Wo_g @ onorm      [1024, S]
Host sums the two group partials per batch, transposes, adds bo.

Perf notes vs v1: query stripes of 1024 (PSUM: s0,s1,pv0,pv1 = 2 banks
each), scores matmuls emitted pairwise so LDWEIGHTS for one 64-row tile
pulls ahead under the other tile's in-flight matmul, exp offloaded
partially to DVE, PSUM evictions split ACT/DVE, and a post-schedule
pass that drops InstLdweights reloading the stationary operand already
resident in the same PE array tile.
"""

import math

import numpy as np
import ml_dtypes

import concourse.bass as bass
import concourse.bacc as bacc_mod
import concourse.mybir as mybir
import concourse.tile as tile
from concourse.bass_utils import run_bass_kernel_spmd

BF16 = mybir.dt.bfloat16
F32 = mybir.dt.float32
I16 = mybir.dt.int16
AF = mybir.ActivationFunctionType
ALU = mybir.AluOpType

B, S, D, H = 4, 2048, 1024, 16
DK = D // H  # 64
NCORES = 8
GROUPS = 2
DG = D // GROUPS  # 512
P = 128
FT = DG // P  # 4 head pairs per group
KB = S // P  # 16 key blocks
DKB = D // P  # 8 contraction blocks
QH = 512  # query stripe
NQH = S // QH  # 4
QC = 512  # matmul free-dim chunk
NQC = S // QC  # 4

# Schraudolph bf16 fast-exp constants (validated on HW: rms 1.8%/tile)
FE_SCALE = 184.66496  # 2^7 * log2(e)
FE_BIAS = 16249.5  # 127*2^7 - 6.5 (error centering)

DEDUP_LDW = True


class _Bacc(bacc_mod.Bacc):
    """Keep semaphore waits on matmuls (split into event-semaphores
    later) instead of migrating them onto ldweights — required for the
    ldweights dedup pass to stay hazard-free."""

    def move_matmul_waits_to_ldweights(self):
        pass


def _row_range(inst):
    tp = inst.tile_position
    ts = inst.tile_size
    r0 = tp[0] if tp is not None else 0
    rn = ts[0] if ts is not None else 128
    return (r0, r0 + rn)


def dedup_ldweights(nc) -> int:
    """Remove InstLdweights whose weights are already resident in the
    targeted PE-array tile (same AP, same tile position/size, no
    row-overlapping load in between).  Only wait-free loads are dropped,
    so synchronization is untouched.  The simulator does not model PE
    weight state; hardware rel-err validates this pass."""
    removed = 0
    for f in nc.m.functions:
        for blk in f.blocks:
            insts = list(blk.instructions)
            keep = []
            changed = False
            loaded: dict = {}  # key -> (sig, row_range)
            for inst in insts:
                if type(inst).__name__ == "InstLdweights":
                    key = (str(inst.tile_position), str(inst.tile_size))
                    sig = str(inst.ins[0])
                    has_wait = bool(inst.sync_info and inst.sync_info.on_wait)
                    prev = loaded.get(key)
                    if not has_wait and prev is not None and prev[0] == sig:
                        removed += 1
                        changed = True
                        continue
                    rr = _row_range(inst)
                    for k in list(loaded):
                        if k != key and _ranges_overlap(loaded[k][1], rr):
                            del loaded[k]
                    loaded[key] = (sig, rr)
                keep.append(inst)
            if changed:
                blk.instructions = keep
    return removed


def _ranges_overlap(a, b):
    return a[0] < b[1] and b[0] < a[1]


def build_attention_nc(seq: int = S) -> bass.Bass:
    kb_n = seq // P
    nqh = seq // QH

    nc = _Bacc("TRN2", num_devices=NCORES)
    xt_d = nc.declare_dram_parameter("xt", [D, seq], BF16, isOutput=False)
    wqt_d = nc.declare_dram_parameter("wqt", [D, DG], BF16, isOutput=False)
    wkt_d = nc.declare_dram_parameter("wkt", [D, DG], BF16, isOutput=False)
    wvt_d = nc.declare_dram_parameter("wvt", [D, DG], BF16, isOutput=False)
    wot_d = nc.declare_dram_parameter("wot", [DG, D], BF16, isOutput=False)
    bq_d = nc.declare_dram_parameter("bqr", [1, DG], BF16, isOutput=False)
    bv_d = nc.declare_dram_parameter("bvr", [1, DG], BF16, isOutput=False)
    out_d = nc.declare_dram_parameter("out", [D, seq], F32, isOutput=True)

    with tile.TileContext(nc) as tc:
        with tc.tile_pool(name="persist", bufs=1) as persist:
            ones_bf = persist.tile([1, QC], BF16, name="ones_bf")
            nc.vector.memset(ones_bf, 1.0)
            bq_sb = persist.tile([1, DG], BF16, name="bq_sb")
            nc.sync.dma_start(bq_sb, bq_d[:, :])
            bv_sb = persist.tile([1, DG], BF16, name="bv_sb")
            nc.sync.dma_start(bv_sb, bv_d[:, :])

            qt_sb = [persist.tile([P, seq], BF16, name=f"qt{i}") for i in range(FT)]
            kt_sb = [persist.tile([P, seq], BF16, name=f"kt{i}") for i in range(FT)]
            v2_sb = [persist.tile([P, 2 * DG], BF16, name=f"v{i}") for i in range(kb_n)]
            wot_sb = [persist.tile([P, D], BF16, name=f"wot{i}") for i in range(FT)]
            onorm = [persist.tile([P, seq], BF16, name=f"onorm{i}") for i in range(FT)]
            for ft in range(FT):
                nc.sync.dma_start(wot_sb[ft], wot_d[ft * P : (ft + 1) * P, :])

            # ---------------- phase 1: projections ----------------
            with tc.tile_pool(name="xw", bufs=1) as xw_pool:
                xt_sb = []
                wq_sb = []
                wk_sb = []
                wv_sb = []
                for i in range(DKB):
                    wq_i = xw_pool.tile([P, DG], BF16, name=f"wqts{i}")
                    nc.sync.dma_start(wq_i, wqt_d[i * P : (i + 1) * P, :])
                    wq_sb.append(wq_i)
                    wk_i = xw_pool.tile([P, DG], BF16, name=f"wkts{i}")
                    nc.sync.dma_start(wk_i, wkt_d[i * P : (i + 1) * P, :])
                    wk_sb.append(wk_i)
                    wv_i = xw_pool.tile([P, DG], BF16, name=f"wvts{i}")
                    nc.sync.dma_start(wv_i, wvt_d[i * P : (i + 1) * P, :])
                    wv_sb.append(wv_i)
                    xti = xw_pool.tile([P, seq], BF16, name=f"xts{i}")
                    nc.sync.dma_start(xti, xt_d[i * P : (i + 1) * P, :])
                    xt_sb.append(xti)

                # Q/K: features on partitions; lhsT reused across chunks
                with tc.tile_pool(name="qkps", bufs=1, space="PSUM") as qk_ps:
                  for pr in range(FT):
                    fsl = slice(pr * P, (pr + 1) * P)
                    psq = [
                        qk_ps.tile([P, QC], F32, name=f"psq{c}", tag=f"psq{c}")
                        for c in range(NQC)
                    ]
                    psk = [
                        qk_ps.tile([P, QC], F32, name=f"psk{c}", tag=f"psk{c}")
                        for c in range(NQC)
                    ]
                    # bias seeds the Q accumulation (per-feature bias via
                    # K=1 matmul: out[f,q] += bq[f] * 1)
                    for c in range(NQC):
                        nc.tensor.matmul(
                            psq[c], lhsT=bq_sb[0:1, fsl], rhs=ones_bf,
                            start=True, stop=False,
                        )
                    for k in range(DKB):
                        for c in range(NQC):
                            csl = slice(c * QC, (c + 1) * QC)
                            nc.tensor.matmul(
                                psq[c], lhsT=wq_sb[k][:, fsl],
                                rhs=xt_sb[k][:, csl],
                                start=False, stop=k == DKB - 1,
                            )
                        for c in range(NQC):
                            csl = slice(c * QC, (c + 1) * QC)
                            nc.tensor.matmul(
                                psk[c], lhsT=wk_sb[k][:, fsl],
                                rhs=xt_sb[k][:, csl],
                                start=k == 0, stop=k == DKB - 1,
                            )
                    for c in range(NQC):
                        csl = slice(c * QC, (c + 1) * QC)
                        nc.scalar.activation(qt_sb[pr][:, csl], psq[c], AF.Identity)
                        nc.scalar.activation(kt_sb[pr][:, csl], psk[c], AF.Identity)

                # V: keys on partitions, features free; bias via K=1 matmul
                with tc.tile_pool(name="vps", bufs=4, space="PSUM") as v_ps:
                    for kb in range(kb_n):
                        ksl = slice(kb * P, (kb + 1) * P)
                        psv = v_ps.tile([P, DG], F32, name="psv", tag="psv")
                        nc.tensor.matmul(
                            psv, lhsT=ones_bf[0:1, 0:P], rhs=bv_sb,
                            start=True, stop=False,
                        )
                        for k in range(DKB):
                            nc.tensor.matmul(
                                psv, lhsT=xt_sb[k][:, ksl], rhs=wv_sb[k],
                                start=False, stop=k == DKB - 1,
                            )
                        nc.vector.memset(v2_sb[kb], 1.0)
                        # even heads -> cols [256q, 0:64); odd -> [256q+192, +256)
                        nc.vector.tensor_copy(
                            v2_sb[kb].rearrange("p (q c) -> p q c", c=256)[:, :, 0:64],
                            psv.rearrange("p (q c) -> p q c", c=128)[:, :, 0:64],
                        )
                        nc.vector.tensor_copy(
                            v2_sb[kb].rearrange("p (q c) -> p q c", c=256)[:, :, 192:256],
                            psv.rearrange("p (q c) -> p q c", c=128)[:, :, 64:128],
                        )

            # ---------------- phase 2: attention ----------------
            # Software-pipelined: scores(kb) and exp(kb) are emitted one
            # iteration ahead of pv(kb) so the PE queue never has a
            # matmul waiting on the exp of scores it just produced
            # (head-of-line blocking starves the PE and drops the HAM
            # clock gate to 1.2 GHz).  s tiles triple-buffered, pv
            # accumulators single (exactly 8 PSUM banks).
            with (
                tc.tile_pool(name="sps", bufs=3, space="PSUM") as s_ps,
                tc.tile_pool(name="pvps", bufs=1, space="PSUM") as pv_ps,
                tc.tile_pool(name="epool", bufs=3) as e_pool,
                tc.tile_pool(name="mpool", bufs=2) as m_pool,
            ):
                for pr in range(FT):
                    h0c = slice((2 * pr) * P, (2 * pr + 1) * P)
                    h1c = slice((2 * pr + 1) * P, (2 * pr + 2) * P)
                    for qh in range(nqh):
                        qbase = qh * QH
                        qsl = slice(qbase, qbase + QH)
                        pv0 = pv_ps.tile([P, QH], F32, name="pv0", tag="pv0")
                        pv1 = pv_ps.tile([P, QH], F32, name="pv1", tag="pv1")
                        es = []

                        def emit_scores(kb):
                            ksl = slice(kb * P, (kb + 1) * P)
                            s0 = s_ps.tile([P, QH], F32, name="s0", tag="s0")
                            s1 = s_ps.tile([P, QH], F32, name="s1", tag="s1")
                            nc.tensor.matmul(
                                s0, lhsT=kt_sb[pr][0:64, ksl],
                                rhs=qt_sb[pr][0:64, qsl],
                                start=True, stop=True,
                            )
                            nc.tensor.matmul(
                                s1, lhsT=kt_sb[pr][64:128, ksl],
                                rhs=qt_sb[pr][64:128, qsl],
                                start=True, stop=True,
                            )
                            e0 = e_pool.tile([P, QH], BF16, name="e0", tag="e0")
                            e1 = e_pool.tile([P, QH], BF16, name="e1", tag="e1")
                            nc.scalar.activation(e0, s0, AF.Exp)
                            nc.vector.tensor_scalar(
                                e1.bitcast(I16), s1, FE_SCALE, FE_BIAS,
                                ALU.mult, ALU.add,
                            )
                            es.append((e0, e1))

                        def emit_pv(kb):
                            e0, e1 = es[kb]
                            first, last = kb == 0, kb == kb_n - 1
                            nc.tensor.matmul(
                                pv0, lhsT=v2_sb[kb][:, h0c], rhs=e0,
                                start=first, stop=last,
                            )
                            nc.tensor.matmul(
                                pv1, lhsT=v2_sb[kb][:, h1c], rhs=e1,
                                start=first, stop=last,
                            )

                        emit_scores(0)
                        for kb in range(1, kb_n):
                            emit_scores(kb)
                            emit_pv(kb - 1)
                        emit_pv(kb_n - 1)

                        # epilogue: evacuate pv (frees banks; split across
                        # ACT/DVE), DMA-swap denominator halves across
                        # partitions, reciprocal, normalize.
                        # pv0 rows 0:64 outT_h0 / 64:128 den_h0 (ones cols)
                        # pv1 rows 0:64 den_h1  / 64:128 outT_h1
                        pvs0 = m_pool.tile([P, QH], F32, name="pvs0", tag="pvs0")
                        pvs1 = m_pool.tile([P, QH], F32, name="pvs1", tag="pvs1")
                        nc.scalar.activation(pvs0, pv0, AF.Identity)
                        nc.vector.tensor_copy(pvs1, pv1)
                        dsw = m_pool.tile([P, QH], F32, name="dsw", tag="dsw")
                        nc.sync.dma_start(dsw[0:64, :], pvs0[64:128, :])
                        nc.sync.dma_start(dsw[64:128, :], pvs1[0:64, :])
                        rec = m_pool.tile([P, QH], F32, name="rec", tag="rec")
                        nc.vector.reciprocal_approx_fast(rec, dsw)
                        nc.vector.tensor_tensor(
                            onorm[pr][0:64, qsl], pvs0[0:64, :], rec[0:64, :],
                            ALU.mult,
                        )
                        nc.vector.tensor_tensor(
                            onorm[pr][64:128, qsl], pvs1[64:128, :], rec[64:128, :],
                            ALU.mult,
                        )

            # ---------------- phase 3: output projection ----------------
            with (
                tc.tile_pool(name="ops", bufs=2, space="PSUM") as o_ps,
                tc.tile_pool(name="osb", bufs=8) as o_sb_pool,
            ):
                for dt in range(DKB):
                    dsl = slice(dt * P, (dt + 1) * P)
                    pso = [
                        o_ps.tile([P, QC], F32, name=f"pso{c}", tag=f"pso{c}")
                        for c in range(NQC)
                    ]
                    for ft in range(FT):
                        for c in range(NQC):
                            csl = slice(c * QC, (c + 1) * QC)
                            nc.tensor.matmul(
                                pso[c], lhsT=wot_sb[ft][:, dsl],
                                rhs=onorm[ft][:, csl],
                                start=ft == 0, stop=ft == FT - 1,
                            )
                    for c in range(NQC):
                        csl = slice(c * QC, (c + 1) * QC)
                        o_sb = o_sb_pool.tile([P, QC], F32, name="o_sb", tag="osb")
                        if c % 2 == 0:
                            nc.scalar.activation(o_sb, pso[c], AF.Identity)
                        else:
                            nc.vector.tensor_copy(o_sb, pso[c])
                        nc.sync.dma_start(out_d[dsl, csl], o_sb)

    return nc


_CACHE: dict = {}


def _get_nc(seq: int = S) -> bass.Bass:
    key = f"nc{seq}"
    if key not in _CACHE:
        nc = build_attention_nc(seq)
        if DEDUP_LDW:
            dedup_ldweights(nc)
        nc.finalize()
        _CACHE[key] = nc
    return _CACHE[key]


def make_in_maps(x, Wq, bq, Wk, Wv, bv, Wo, seq: int = S):
    bf = ml_dtypes.bfloat16
    scale = 1.0 / math.sqrt(DK)
    x = np.asarray(x, np.float32)
    Wq = np.asarray(Wq, np.float32)
    bq = np.asarray(bq, np.float32)
    Wk = np.asarray(Wk, np.float32)
    Wv = np.asarray(Wv, np.float32)
    bv = np.asarray(bv, np.float32)
    Wo = np.asarray(Wo, np.float32)
    in_maps = []
    for core in range(NCORES):
        b, g = divmod(core, GROUPS)
        gsl = slice(g * DG, (g + 1) * DG)
        in_maps.append(
            {
                "xt": np.ascontiguousarray(x[b, :seq, :].T).astype(bf),
                "wqt": np.ascontiguousarray((Wq[gsl, :] * scale).T).astype(bf),
                "wkt": np.ascontiguousarray(Wk[gsl, :].T).astype(bf),
                "wvt": np.ascontiguousarray(Wv[gsl, :].T).astype(bf),
                "wot": np.ascontiguousarray(Wo[:, gsl].T).astype(bf),
                "bqr": (bq[gsl] * scale).astype(bf).reshape(1, DG),
                "bvr": bv[gsl].astype(bf).reshape(1, DG),
            }
        )
    return in_maps


def run_device(in_maps, seq: int = S, trace: bool = False):
    nc = _get_nc(seq)
    return run_bass_kernel_spmd(nc, in_maps, list(range(NCORES)), trace=trace)


def kernel(x, Wq, bq, Wk, bk, Wv, bv, Wo, bo):
    in_maps = make_in_maps(x, Wq, bq, Wk, Wv, bv, Wo)
    res = run_device(in_maps).results
    bo = np.asarray(bo, np.float32)
    out = np.empty((B, S, D), np.float32)
    for b in range(B):
        acc = res[2 * b]["out"].astype(np.float32) + res[2 * b + 1]["out"].astype(
            np.float32
        )
        out[b] = acc.T + bo[None, :]
    return out


# revision 13
# speedup vs baseline: 1.3556x; 1.0552x over previous
"""Multi-head self-attention on 8 Trainium2 NeuronCores — v2.

Problem: B=4, S=2048, D=1024, H=16 heads (dk=64), torch-Linear style
projections (y = x @ W.T + b), softmax attention, output projection.

Sharding: 8 cores = 4 batches x 2 head-groups (8 heads each).  Per core:
    QT = (Wq_g/sqrt(dk)) @ x_b.T (+bq via K=1 ones-matmul)  [512, S]
    KT = Wk_g @ x_b.T            [512, S]  (bk dropped: cancels in softmax)
    V  = x_b @ Wv_g.T + bv       [S, 512]
    per head-pair pr: scoresT = K_h @ Q_h.T  as TWO concurrent 64-row
    PE tiles (row tiling: head even on partitions 0:64 / tile (0,0),
    head odd on 64:128 / tile (64,0)) writing separate PSUM banks.
    exp: head-even tiles on the Scalar engine (exact exp), a tunable
    subset of head-odd tiles on the Vector engine via a one-instruction
    Schraudolph fast-exp (bits = int16(s*184.66496 + 16249.5) viewed as
    bf16; ~1.8% rms per affected tile, diluted by the subset fraction).
    PV via the ones-columns trick ([V_h|ones] / [ones|V_h] stationaries)
    so softmax denominators fall out of the same matmuls.
    partialT = Wo_g @ onorm      [1024, S]
Host sums the two group partials per batch, transposes, adds bo.

Perf notes vs v1: query stripes of 1024 (PSUM: s0,s1,pv0,pv1 = 2 banks
each), scores matmuls emitted pairwise so LDWEIGHTS for one 64-row tile
pulls ahead under the other tile's in-flight matmul, exp offloaded
partially to DVE, PSUM evictions split ACT/DVE, and a post-schedule
pass that drops InstLdweights reloading the stationary operand already
resident in the same PE array tile.
"""

import math

import numpy as np
import ml_dtypes

import concourse.bass as bass
import concourse.bacc as bacc_mod
import concourse.mybir as mybir
import concourse.tile as tile
from concourse.bass_utils import run_bass_kernel_spmd

BF16 = mybir.dt.bfloat16
F32 = mybir.dt.float32
I16 = mybir.dt.int16
AF = mybir.ActivationFunctionType
ALU = mybir.AluOpType

B, S, D, H = 4, 2048, 1024, 16
DK = D // H  # 64
NCORES = 8
GROUPS = 2
DG = D // GROUPS  # 512
P = 128
FT = DG // P  # 4 head pairs per group
KB = S // P  # 16 key blocks
DKB = D // P  # 8 contraction blocks
QH = 512  # query stripe
NQH = S // QH  # 4
QC = 512  # matmul free-dim chunk
NQC = S // QC  # 4

# Schraudolph bf16 fast-exp constants (validated on HW: rms 1.8%/tile)
FE_SCALE = 184.66496  # 2^7 * log2(e)
FE_BIAS = 16249.5  # 127*2^7 - 6.5 (error centering)
# head-odd exp tiles routed to ACT instead of DVE (engine balancing)
E1_ACT_KBS = frozenset({0, 5, 10})

DEDUP_LDW = True


class _Bacc(bacc_mod.Bacc):
    """Keep semaphore waits on matmuls (split into event-semaphores
    later) instead of migrating them onto ldweights — required for the
    ldweights dedup pass to stay hazard-free."""

    def move_matmul_waits_to_ldweights(self):
        pass


def _row_range(inst):
    tp = inst.tile_position
    ts = inst.tile_size
    r0 = tp[0] if tp is not None else 0
    rn = ts[0] if ts is not None else 128
    return (r0, r0 + rn)


def dedup_ldweights(nc) -> int:
    """Remove InstLdweights whose weights are already resident in the
    targeted PE-array tile (same AP, same tile position/size, no
    row-overlapping load in between).  Only wait-free loads are dropped,
    so synchronization is untouched.  The simulator does not model PE
    weight state; hardware rel-err validates this pass."""
    removed = 0
    for f in nc.m.functions:
        for blk in f.blocks:
            insts = list(blk.instructions)
            keep = []
            changed = False
            loaded: dict = {}  # key -> (sig, row_range)
            for inst in insts:
                if type(inst).__name__ == "InstLdweights":
                    key = (str(inst.tile_position), str(inst.tile_size))
                    sig = str(inst.ins[0])
                    has_wait = bool(inst.sync_info and inst.sync_info.on_wait)
                    prev = loaded.get(key)
                    if not has_wait and prev is not None and prev[0] == sig:
                        removed += 1
                        changed = True
                        continue
                    rr = _row_range(inst)
                    for k in list(loaded):
                        if k != key and _ranges_overlap(loaded[k][1], rr):
                            del loaded[k]
                    loaded[key] = (sig, rr)
                keep.append(inst)
            if changed:
                blk.instructions = keep
    return removed


def _ranges_overlap(a, b):
    return a[0] < b[1] and b[0] < a[1]


def build_attention_nc(seq: int = S) -> bass.Bass:
    kb_n = seq // P
    nqh = seq // QH

    nc = _Bacc("TRN2", num_devices=NCORES)
    xt_d = nc.declare_dram_parameter("xt", [D, seq], BF16, isOutput=False)
    wqt_d = nc.declare_dram_parameter("wqt", [D, DG], BF16, isOutput=False)
    wkt_d = nc.declare_dram_parameter("wkt", [D, DG], BF16, isOutput=False)
    wvt_d = nc.declare_dram_parameter("wvt", [D, DG], BF16, isOutput=False)
    wot_d = nc.declare_dram_parameter("wot", [DG, D], BF16, isOutput=False)
    bq_d = nc.declare_dram_parameter("bqs", [P, FT], F32, isOutput=False)
    bv_d = nc.declare_dram_parameter("bvr", [1, DG], BF16, isOutput=False)
    out_d = nc.declare_dram_parameter("out", [D, seq], F32, isOutput=True)

    with tile.TileContext(nc) as tc:
        with tc.tile_pool(name="persist", bufs=1) as persist:
            ones_bf = persist.tile([1, QC], BF16, name="ones_bf")
            nc.vector.memset(ones_bf, 1.0)
            bq_sb = persist.tile([P, FT], F32, name="bq_sb")
            nc.sync.dma_start(bq_sb, bq_d[:, :])
            bv_sb = persist.tile([1, DG], BF16, name="bv_sb")
            nc.sync.dma_start(bv_sb, bv_d[:, :])

            qt_sb = [persist.tile([P, seq], BF16, name=f"qt{i}") for i in range(FT)]
            kt_sb = [persist.tile([P, seq], BF16, name=f"kt{i}") for i in range(FT)]
            v2_sb = [persist.tile([P, 2 * DG], BF16, name=f"v{i}") for i in range(kb_n)]
            wot_sb = [persist.tile([P, D], BF16, name=f"wot{i}") for i in range(FT)]
            onorm = [persist.tile([P, seq], BF16, name=f"onorm{i}") for i in range(FT)]
            for ft in range(FT):
                nc.sync.dma_start(wot_sb[ft], wot_d[ft * P : (ft + 1) * P, :])

            # ---------------- phase 1: projections ----------------
            with tc.tile_pool(name="xw", bufs=1) as xw_pool:
                xt_sb = []
                wq_sb = []
                wk_sb = []
                wv_sb = []
                for i in range(DKB):
                    wq_i = xw_pool.tile([P, DG], BF16, name=f"wqts{i}")
                    nc.sync.dma_start(wq_i, wqt_d[i * P : (i + 1) * P, :])
                    wq_sb.append(wq_i)
                    wk_i = xw_pool.tile([P, DG], BF16, name=f"wkts{i}")
                    nc.sync.dma_start(wk_i, wkt_d[i * P : (i + 1) * P, :])
                    wk_sb.append(wk_i)
                    wv_i = xw_pool.tile([P, DG], BF16, name=f"wvts{i}")
                    nc.sync.dma_start(wv_i, wvt_d[i * P : (i + 1) * P, :])
                    wv_sb.append(wv_i)
                    xti = xw_pool.tile([P, seq], BF16, name=f"xts{i}")
                    nc.sync.dma_start(xti, xt_d[i * P : (i + 1) * P, :])
                    xt_sb.append(xti)

                # Q/K: features on partitions; lhsT reused across chunks
                with tc.tile_pool(name="qkps", bufs=1, space="PSUM") as qk_ps:
                  for pr in range(FT):
                    fsl = slice(pr * P, (pr + 1) * P)
                    psq = [
                        qk_ps.tile([P, QC], F32, name=f"psq{c}", tag=f"psq{c}")
                        for c in range(NQC)
                    ]
                    psk = [
                        qk_ps.tile([P, QC], F32, name=f"psk{c}", tag=f"psk{c}")
                        for c in range(NQC)
                    ]
                    for k in range(DKB):
                        for c in range(NQC):
                            csl = slice(c * QC, (c + 1) * QC)
                            nc.tensor.matmul(
                                psq[c], lhsT=wq_sb[k][:, fsl],
                                rhs=xt_sb[k][:, csl],
                                start=k == 0, stop=k == DKB - 1,
                            )
                        for c in range(NQC):
                            csl = slice(c * QC, (c + 1) * QC)
                            nc.tensor.matmul(
                                psk[c], lhsT=wk_sb[k][:, fsl],
                                rhs=xt_sb[k][:, csl],
                                start=k == 0, stop=k == DKB - 1,
                            )
                    for c in range(NQC):
                        csl = slice(c * QC, (c + 1) * QC)
                        nc.scalar.activation(
                            qt_sb[pr][:, csl], psq[c], AF.Identity,
                            bias=bq_sb[:, pr : pr + 1],
                        )
                        nc.scalar.activation(kt_sb[pr][:, csl], psk[c], AF.Identity)

                # V: keys on partitions, features free; bias via K=1 matmul
                with tc.tile_pool(name="vps", bufs=4, space="PSUM") as v_ps:
                    for kb in range(kb_n):
                        ksl = slice(kb * P, (kb + 1) * P)
                        psv = v_ps.tile([P, DG], F32, name="psv", tag="psv")
                        nc.tensor.matmul(
                            psv, lhsT=ones_bf[0:1, 0:P], rhs=bv_sb,
                            start=True, stop=False,
                        )
                        for k in range(DKB):
                            nc.tensor.matmul(
                                psv, lhsT=xt_sb[k][:, ksl], rhs=wv_sb[k],
                                start=False, stop=k == DKB - 1,
                            )
                        nc.vector.memset(v2_sb[kb], 1.0)
                        # even heads -> cols [256q, 0:64); odd -> [256q+192, +256)
                        nc.vector.tensor_copy(
                            v2_sb[kb].rearrange("p (q c) -> p q c", c=256)[:, :, 0:64],
                            psv.rearrange("p (q c) -> p q c", c=128)[:, :, 0:64],
                        )
                        nc.vector.tensor_copy(
                            v2_sb[kb].rearrange("p (q c) -> p q c", c=256)[:, :, 192:256],
                            psv.rearrange("p (q c) -> p q c", c=128)[:, :, 64:128],
                        )

            # ---------------- phase 2: attention ----------------
            # Two query stripes (A/B, 512 each) processed per block so
            # the scores/PV stationaries are loaded once per two
            # matmuls (ldweights dedup removes the reloads), scores and
            # exp run one kb ahead of PV (no PE head-of-line blocking),
            # and exp runs on [128,1024] tiles to amortize the ~300ns
            # fixed per-op engine overhead.  PSUM: sAB0,sAB1 (2 banks
            # each) + pvA0,pvA1,pvB0,pvB1 (1 each) = 8 banks.
            with (
                tc.tile_pool(name="sps", bufs=1, space="PSUM") as s_ps,
                tc.tile_pool(name="pvps", bufs=1, space="PSUM") as pv_ps,
                tc.tile_pool(name="epool", bufs=3) as e_pool,
                tc.tile_pool(name="mpool", bufs=2) as m_pool,
            ):
                for pr in range(FT):
                    h0c = slice((2 * pr) * P, (2 * pr + 1) * P)
                    h1c = slice((2 * pr + 1) * P, (2 * pr + 2) * P)
                    for qp in range(nqh // 2):
                        qbase = qp * 2 * QC
                        qA = slice(qbase, qbase + QC)
                        qB = slice(qbase + QC, qbase + 2 * QC)
                        pvA0 = pv_ps.tile([P, QC], F32, name="pvA0", tag="pvA0")
                        pvA1 = pv_ps.tile([P, QC], F32, name="pvA1", tag="pvA1")
                        pvB0 = pv_ps.tile([P, QC], F32, name="pvB0", tag="pvB0")
                        pvB1 = pv_ps.tile([P, QC], F32, name="pvB1", tag="pvB1")
                        es = []

                        def emit_scores(kb):
                            ksl = slice(kb * P, (kb + 1) * P)
                            s0 = s_ps.tile([P, 2 * QC], F32, name="s0", tag="s0")
                            s1 = s_ps.tile([P, 2 * QC], F32, name="s1", tag="s1")
                            # emission order: same lhsT twice per head so
                            # the reload dedups; h0/h64 tiles interleave so
                            # the pairs stream concurrently
                            nc.tensor.matmul(
                                s0[:, 0:QC], lhsT=kt_sb[pr][0:64, ksl],
                                rhs=qt_sb[pr][0:64, qA], start=True, stop=True,
                            )
                            nc.tensor.matmul(
                                s1[:, 0:QC], lhsT=kt_sb[pr][64:128, ksl],
                                rhs=qt_sb[pr][64:128, qA], start=True, stop=True,
                            )
                            nc.tensor.matmul(
                                s0[:, QC : 2 * QC], lhsT=kt_sb[pr][0:64, ksl],
                                rhs=qt_sb[pr][0:64, qB], start=True, stop=True,
                            )
                            nc.tensor.matmul(
                                s1[:, QC : 2 * QC], lhsT=kt_sb[pr][64:128, ksl],
                                rhs=qt_sb[pr][64:128, qB], start=True, stop=True,
                            )
                            e0 = e_pool.tile([P, 2 * QC], BF16, name="e0", tag="e0")
                            e1 = e_pool.tile([P, 2 * QC], BF16, name="e1", tag="e1")
                            nc.scalar.activation(e0, s0, AF.Exp)
                            if kb in E1_ACT_KBS:
                                nc.scalar.activation(e1, s1, AF.Exp)
                            else:
                                nc.vector.tensor_scalar(
                                    e1.bitcast(I16), s1, FE_SCALE, FE_BIAS,
                                    ALU.mult, ALU.add,
                                )
                            es.append((e0, e1))

                        def emit_pv(kb):
                            e0, e1 = es[kb]
                            first, last = kb == 0, kb == kb_n - 1
                            nc.tensor.matmul(
                                pvA0, lhsT=v2_sb[kb][:, h0c], rhs=e0[:, 0:QC],
                                start=first, stop=last,
                            )
                            nc.tensor.matmul(
                                pvB0, lhsT=v2_sb[kb][:, h0c],
                                rhs=e0[:, QC : 2 * QC], start=first, stop=last,
                            )
                            nc.tensor.matmul(
                                pvA1, lhsT=v2_sb[kb][:, h1c], rhs=e1[:, 0:QC],
                                start=first, stop=last,
                            )
                            nc.tensor.matmul(
                                pvB1, lhsT=v2_sb[kb][:, h1c],
                                rhs=e1[:, QC : 2 * QC], start=first, stop=last,
                            )

                        emit_scores(0)
                        for kb in range(1, kb_n):
                            emit_scores(kb)
                            emit_pv(kb - 1)
                        emit_pv(kb_n - 1)

                        # epilogue (both stripes at once): evacuate pv
                        # (ACT for h0, DVE for h1; frees banks), DMA-swap
                        # denominator halves across partitions,
                        # reciprocal, normalize.
                        # pv*0 rows 0:64 outT_h0 / 64:128 den_h0
                        # pv*1 rows 0:64 den_h1  / 64:128 outT_h1
                        pvs0 = m_pool.tile([P, 2 * QC], F32, name="pvs0", tag="pvs0")
                        pvs1 = m_pool.tile([P, 2 * QC], F32, name="pvs1", tag="pvs1")
                        nc.scalar.activation(pvs0[:, 0:QC], pvA0, AF.Identity)
                        nc.scalar.activation(pvs0[:, QC : 2 * QC], pvB0, AF.Identity)
                        nc.vector.tensor_copy(pvs1[:, 0:QC], pvA1)
                        nc.vector.tensor_copy(pvs1[:, QC : 2 * QC], pvB1)
                        dsw = m_pool.tile([P, 2 * QC], F32, name="dsw", tag="dsw")
                        nc.sync.dma_start(dsw[0:64, :], pvs0[64:128, :])
                        nc.sync.dma_start(dsw[64:128, :], pvs1[0:64, :])
                        rec = m_pool.tile([P, 2 * QC], F32, name="rec", tag="rec")
                        nc.vector.reciprocal_approx_fast(rec, dsw)
                        qful = slice(qbase, qbase + 2 * QC)
                        nc.vector.tensor_tensor(
                            onorm[pr][0:64, qful], pvs0[0:64, :], rec[0:64, :],
                            ALU.mult,
                        )
                        nc.vector.tensor_tensor(
                            onorm[pr][64:128, qful], pvs1[64:128, :], rec[64:128, :],
                            ALU.mult,
                        )

            # ---------------- phase 3: output projection ----------------
            with (
                tc.tile_pool(name="ops", bufs=2, space="PSUM") as o_ps,
                tc.tile_pool(name="osb", bufs=8) as o_sb_pool,
            ):
                for dt in range(DKB):
                    dsl = slice(dt * P, (dt + 1) * P)
                    pso = [
                        o_ps.tile([P, QC], F32, name=f"pso{c}", tag=f"pso{c}")
                        for c in range(NQC)
                    ]
                    for ft in range(FT):
                        for c in range(NQC):
                            csl = slice(c * QC, (c + 1) * QC)
                            nc.tensor.matmul(
                                pso[c], lhsT=wot_sb[ft][:, dsl],
                                rhs=onorm[ft][:, csl],
                                start=ft == 0, stop=ft == FT - 1,
                            )
                    for c in range(NQC):
                        csl = slice(c * QC, (c + 1) * QC)
                        o_sb = o_sb_pool.tile([P, QC], F32, name="o_sb", tag="osb")
                        if c % 2 == 0:
                            nc.scalar.activation(o_sb, pso[c], AF.Identity)
                        else:
                            nc.vector.tensor_copy(o_sb, pso[c])
                        nc.sync.dma_start(out_d[dsl, csl], o_sb)

    return nc


_CACHE: dict = {}


def _get_nc(seq: int = S) -> bass.Bass:
    key = f"nc{seq}"
    if key not in _CACHE:
        nc = build_attention_nc(seq)
        if DEDUP_LDW:
            dedup_ldweights(nc)
        nc.finalize()
        _CACHE[key] = nc
    return _CACHE[key]


def make_in_maps(x, Wq, bq, Wk, Wv, bv, Wo, seq: int = S):
    bf = ml_dtypes.bfloat16
    scale = 1.0 / math.sqrt(DK)
    x = np.asarray(x, np.float32)
    Wq = np.asarray(Wq, np.float32)
    bq = np.asarray(bq, np.float32)
    Wk = np.asarray(Wk, np.float32)
    Wv = np.asarray(Wv, np.float32)
    bv = np.asarray(bv, np.float32)
    Wo = np.asarray(Wo, np.float32)
    in_maps = []
    for core in range(NCORES):
        b, g = divmod(core, GROUPS)
        gsl = slice(g * DG, (g + 1) * DG)
        in_maps.append(
            {
                "xt": np.ascontiguousarray(x[b, :seq, :].T).astype(bf),
                "wqt": np.ascontiguousarray((Wq[gsl, :] * scale).T).astype(bf),
                "wkt": np.ascontiguousarray(Wk[gsl, :].T).astype(bf),
                "wvt": np.ascontiguousarray(Wv[gsl, :].T).astype(bf),
                "wot": np.ascontiguousarray(Wo[:, gsl].T).astype(bf),
                "bqs": np.ascontiguousarray(
                    (bq[gsl] * scale).astype(np.float32).reshape(FT, P).T
                ),
                "bvr": bv[gsl].astype(bf).reshape(1, DG),
            }
        )
    return in_maps


def run_device(in_maps, seq: int = S, trace: bool = False):
    nc = _get_nc(seq)
    return run_bass_kernel_spmd(nc, in_maps, list(range(NCORES)), trace=trace)


def kernel(x, Wq, bq, Wk, bk, Wv, bv, Wo, bo):
    in_maps = make_in_maps(x, Wq, bq, Wk, Wv, bv, Wo)
    res = run_device(in_maps).results
    bo = np.asarray(bo, np.float32)
    out = np.empty((B, S, D), np.float32)
    for b in range(B):
        acc = res[2 * b]["out"].astype(np.float32) + res[2 * b + 1]["out"].astype(
            np.float32
        )
        out[b] = acc.T + bo[None, :]
    return out
